# revision 11
# baseline (speedup 1.0000x reference)
"""Trainium2 Bass kernel for an additive-attention module.

Computes (matching the PyTorch/JAX reference):
    energy  = tanh(enc @ We + h @ Wh + b)      # (S, B, 1)
    attn    = softmax(energy, axis=0)          # softmax over the S axis
    context = sum_s attn[s, b] * enc[s, b, :]  # (B, D)
    out     = context[None]                    # (1, B, D)

Shapes (hardcoded): enc (512, 512, 512) f32, hidden (512, 512) f32,
W (1024, 1) f32, b (1,) f32.

Sharding: pure data parallel over dim 1 (the batch axis) across 8
NeuronCores; softmax axis (dim 0) stays local per shard. W/b replicated.
No collectives.

Per-core algorithm (B_LOC = 64 batch columns per core):
  - host precomputes hb[b] = hidden[b] @ Wh + bias  (tiny)
  - layout: s on partitions (4 chunks of 128), (b, d) on the free dim
  - energy: one fused DVE tensor_tensor_reduce per (b, s-chunk):
        accum[s] = hb[b] + sum_d enc[s,b,d] * We[d]
  - tanh+exp on ScalarE (values in (-1,1) so softmax needs no max-sub)
  - denominator: free-dim reduce on DVE then a ones-matmul on PE
  - context: PE matmul, lhsT = p column (128,1), rhs = enc tile
    (128,512), accumulated over the 4 s-chunks in PSUM (float32r: full
    rate; products are ~bf16-rounded which is well inside tolerance)
  - normalize on ScalarE (scale=1/denom), batched output DMA
"""

import os
import sys

import numpy as np

S = 512
D = 512
N_CORES = 8
B_LOC = S // N_CORES        # 64 batch columns per core
NB = 4                      # batch columns per group (inner tiling)
P = 128                     # SBUF partitions
N_CHUNK = S // P            # 4 s-chunks


def _import_concourse():
    try:
        import concourse  # noqa: F401
    except ImportError:
        sys.path.insert(0, "/opt/trn_rl_repo")
        import concourse  # noqa: F401


_NC_CACHE = {}


def _build_nc(b_loc=B_LOC):
    """Build and compile the (single-core SPMD) bass program."""
    _import_concourse()
    import concourse.bacc as bacc
    import concourse.tile as tile
    from concourse import mybir

    f32 = mybir.dt.float32
    bf16 = mybir.dt.bfloat16
    Alu = mybir.AluOpType
    Act = mybir.ActivationFunctionType

    nc = bacc.Bacc(
        trn_type="TRN2",
        target_bir_lowering=False,
        debug=False,
        enable_asserts=True,
        num_devices=N_CORES,
    )

    enc = nc.dram_tensor("enc", (S, b_loc, D), bf16, kind="ExternalInput").ap()
    web = nc.dram_tensor("web", (D,), bf16, kind="ExternalInput").ap()
    hbb = nc.dram_tensor("hbb", (b_loc,), f32, kind="ExternalInput").ap()
    out = nc.dram_tensor("out", (b_loc, D), f32, kind="ExternalOutput").ap()

    n_groups = b_loc // NB

    with tile.TileContext(nc) as tc:
        with (
            tc.tile_pool(name="consts", bufs=1) as consts,
            tc.tile_pool(name="encp", bufs=16) as encp,
            tc.tile_pool(name="scrp", bufs=3) as scrp,
            tc.tile_pool(name="small", bufs=24) as small,
            tc.tile_pool(name="ost", bufs=3) as ost,
            tc.tile_pool(name="pctx", bufs=4, space="PSUM") as pctx,
            tc.tile_pool(name="pden", bufs=3, space="PSUM") as pden,
        ):
            we_t = consts.tile([P, D], bf16)
            nc.sync.dma_start(out=we_t, in_=web.partition_broadcast(P))
            hb_t = consts.tile([P, b_loc], f32)
            nc.sync.dma_start(out=hb_t, in_=hbb.partition_broadcast(P))
            ones_t = consts.tile([P, 1], f32)
            nc.vector.memset(ones_t, 1.0)

            for g in range(n_groups):
                enc_tiles = []
                for c in range(N_CHUNK):
                    t = encp.tile([P, NB, D], bf16)
                    nc.sync.dma_start(
                        out=t,
                        in_=enc[c * P:(c + 1) * P, g * NB:(g + 1) * NB, :],
                    )
                    enc_tiles.append(t)

                out_t = ost.tile([1, NB * D], f32)

                for j in range(NB):
                    b = g * NB + j
                    # energy columns: e_cols[s, c] = enc[s,b,:] @ We via the
                    # fused DVE multiply+accumulate (single pass; bf16 dense
                    # operands keep the DVE 2x packed mode eligible, so the
                    # product stream goes to a real dense scratch tile)
                    e_cols = small.tile([P, N_CHUNK], f32)
                    for c in range(N_CHUNK):
                        scr = scrp.tile([P, D], bf16)
                        nc.vector.scalar_tensor_tensor(
                            out=scr,
                            in0=enc_tiles[c][:, j, :],
                            scalar=1.0,
                            in1=we_t,
                            op0=Alu.mult,
                            op1=Alu.mult,
                            accum_out=e_cols[:, c:c + 1],
                        )
                    # p = exp(tanh(e + hb[b]))  (hb folded into the Tanh
                    # bias; both functions live in the exp table set);
                    # p in bf16 so the context matmul runs at full PE rate
                    t_t = small.tile([P, N_CHUNK], f32)
                    nc.scalar.activation(out=t_t, in_=e_cols, func=Act.Tanh,
                                         bias=hb_t[:, b:b + 1])
                    p_t = small.tile([P, N_CHUNK], bf16)
                    nc.scalar.activation(out=p_t, in_=t_t, func=Act.Exp)

                    # denominator: sum over all 512 s values (fp32 matmul
                    # with N=1 is cheap; ones.T @ prs contracts partitions)
                    prs = small.tile([P, 1], f32)
                    nc.vector.reduce_sum(
                        out=prs, in_=p_t, axis=mybir.AxisListType.X
                    )
                    den_ps = pden.tile([1, 1], f32)
                    nc.tensor.matmul(
                        out=den_ps,
                        lhsT=ones_t,
                        rhs=prs,
                        start=True,
                        stop=True,
                    )
                    rec = small.tile([1, 1], f32)
                    nc.vector.reciprocal(out=rec, in_=den_ps)

                    # context: sum_s p[s] * enc[s, b, :] via PE accumulation
                    # (dense bf16 operands, fp32 PSUM accumulate)
                    ctx_ps = pctx.tile([1, D], f32)
                    for c in range(N_CHUNK):
                        nc.tensor.matmul(
                            out=ctx_ps,
                            lhsT=p_t[:, c:c + 1],
                            rhs=enc_tiles[c][:, j, :],
                            start=(c == 0),
                            stop=(c == N_CHUNK - 1),
                        )
                    # normalize: out_row = ctx / denom
                    nc.scalar.activation(
                        out=out_t[:, j * D:(j + 1) * D],
                        in_=ctx_ps,
                        func=Act.Copy,
                        scale=rec,
                    )

                nc.sync.dma_start(
                    out=out[g * NB:(g + 1) * NB, :],
                    in_=out_t,
                )

    nc.compile()
    return nc


def _get_nc(b_loc=B_LOC):
    if b_loc not in _NC_CACHE:
        _NC_CACHE[b_loc] = _build_nc(b_loc)
    return _NC_CACHE[b_loc]


def _host_prep(encoder_outputs, hidden_state, W, b):
    import ml_dtypes

    enc = np.asarray(encoder_outputs, dtype=np.float32)
    h = np.asarray(hidden_state, dtype=np.float32)
    W = np.asarray(W, dtype=np.float32)
    b = np.asarray(b, dtype=np.float32)
    # device-side compute streams enc in bf16 (round-to-nearest); the
    # softmax weights and normalization stay fp32 on device
    enc_bf = enc.astype(ml_dtypes.bfloat16)
    We = np.ascontiguousarray(W[:D, 0]).astype(ml_dtypes.bfloat16)
    Wh = W[D:, 0]
    hb = (h @ Wh + b[0]).astype(np.float32)  # (S,)
    return enc_bf, We, hb


def kernel(encoder_outputs, hidden_state, W, b):
    enc, We, hb = _host_prep(encoder_outputs, hidden_state, W, b)
    nc = _get_nc()

    from concourse import bass_utils

    in_maps = []
    for c in range(N_CORES):
        sl = slice(c * B_LOC, (c + 1) * B_LOC)
        in_maps.append({
            "enc": np.ascontiguousarray(enc[:, sl, :]),
            "web": We,
            "hbb": np.ascontiguousarray(hb[sl]),
        })

    res = bass_utils.run_bass_kernel_spmd(
        nc,
        in_maps,
        core_ids=list(range(N_CORES)),
        trace=bool(int(os.environ.get("KERNEL_TRACE", "0"))),
    )
    global LAST_RESULTS
    LAST_RESULTS = res
    ctx = np.concatenate([r["out"] for r in res.results], axis=0)  # (S, D)
    return ctx[None].astype(np.float32)


LAST_RESULTS = None


# revision 16
# speedup vs baseline: 1.1363x; 1.1363x over previous
"""Trainium2 Bass kernel for an additive-attention module.

Computes (matching the PyTorch/JAX reference):
    energy  = tanh(enc @ We + h @ Wh + b)      # (S, B, 1)
    attn    = softmax(energy, axis=0)          # softmax over the S axis
    context = sum_s attn[s, b] * enc[s, b, :]  # (B, D)
    out     = context[None]                    # (1, B, D)

Shapes (hardcoded): enc (512, 512, 512) f32, hidden (512, 512) f32,
W (1024, 1) f32, b (1,) f32.

Sharding: pure data parallel over dim 1 (the batch axis) across 8
NeuronCores; softmax axis (dim 0) stays local per shard. W/b replicated.
No collectives.

Per-core algorithm (B_LOC = 64 batch columns per core):
  - host precomputes hb[b] = hidden[b] @ Wh + bias  (tiny)
  - layout: s on partitions (4 chunks of 128), (b, d) on the free dim
  - energy: one fused DVE tensor_tensor_reduce per (b, s-chunk):
        accum[s] = hb[b] + sum_d enc[s,b,d] * We[d]
  - tanh+exp on ScalarE (values in (-1,1) so softmax needs no max-sub)
  - denominator: free-dim reduce on DVE then a ones-matmul on PE
  - context: PE matmul, lhsT = p column (128,1), rhs = enc tile
    (128,512), accumulated over the 4 s-chunks in PSUM (float32r: full
    rate; products are ~bf16-rounded which is well inside tolerance)
  - normalize on ScalarE (scale=1/denom), batched output DMA
"""

import os
import sys

import numpy as np

S = 512
D = 512
N_CORES = 8
B_LOC = S // N_CORES        # 64 batch columns per core
NB = 4                      # batch columns per group (inner tiling)
P = 128                     # SBUF partitions
N_CHUNK = S // P            # 4 s-chunks


def _import_concourse():
    try:
        import concourse  # noqa: F401
    except ImportError:
        sys.path.insert(0, "/opt/trn_rl_repo")
        import concourse  # noqa: F401


_NC_CACHE = {}


def _build_nc(b_loc=B_LOC):
    """Build and compile the (single-core SPMD) bass program."""
    _import_concourse()
    import concourse.bacc as bacc
    import concourse.tile as tile
    from concourse import mybir

    f32 = mybir.dt.float32
    bf16 = mybir.dt.bfloat16
    Alu = mybir.AluOpType
    Act = mybir.ActivationFunctionType

    nc = bacc.Bacc(
        trn_type="TRN2",
        target_bir_lowering=False,
        debug=False,
        enable_asserts=True,
        num_devices=N_CORES,
    )

    enc = nc.dram_tensor("enc", (S, b_loc, D), bf16, kind="ExternalInput").ap()
    web = nc.dram_tensor("web", (NB * D,), bf16, kind="ExternalInput").ap()
    hbb = nc.dram_tensor("hbb", (b_loc,), f32, kind="ExternalInput").ap()
    out = nc.dram_tensor("out", (b_loc, D), f32, kind="ExternalOutput").ap()

    n_groups = b_loc // NB

    with tile.TileContext(nc) as tc:
        with (
            tc.tile_pool(name="consts", bufs=1) as consts,
            tc.tile_pool(name="encp", bufs=16) as encp,
            tc.tile_pool(name="scrp", bufs=6) as scrp,
            tc.tile_pool(name="small", bufs=24) as small,
            tc.tile_pool(name="ost", bufs=3) as ost,
            tc.tile_pool(name="pctx", bufs=5, space="PSUM") as pctx,
            tc.tile_pool(name="pden", bufs=2, space="PSUM") as pden,
        ):
            # We replicated NB times so the group-wide multiply is one dense
            # bf16 op (eligible for the DVE 2x packed mode)
            we_t = consts.tile([P, NB * D], bf16)
            nc.sync.dma_start(out=we_t, in_=web.partition_broadcast(P))
            hb_t = consts.tile([P, b_loc], f32)
            nc.sync.dma_start(out=hb_t, in_=hbb.partition_broadcast(P))
            ones_t = consts.tile([P, 1], f32)
            nc.vector.memset(ones_t, 1.0)

            # reduce work is split between VectorE and ScalarE to balance
            # their busy time; (j, c) pairs with index < split go to DVE
            RED_SPLIT = 7  # of 16 per group

            for g in range(n_groups):
                enc_tiles = []
                for c in range(N_CHUNK):
                    t = encp.tile([P, NB * D], bf16)
                    nc.sync.dma_start(
                        out=t,
                        in_=enc[c * P:(c + 1) * P, g * NB:(g + 1) * NB, :],
                    )
                    enc_tiles.append(t)

                out_t = ost.tile([1, NB * D], f32)

                # phase 1: products prod[s, (b,d)] = enc * We  (DVE 2x)
                prods = []
                for c in range(N_CHUNK):
                    tmp = scrp.tile([P, NB * D], bf16)
                    nc.vector.tensor_mul(tmp, enc_tiles[c], we_t)
                    prods.append(tmp)

                # phase 2: energy columns e[s, j*NC+c] = sum_d prod, split
                # across DVE tensor_reduce and ScalarE activation-accumulate
                e_cols = small.tile([P, NB * N_CHUNK], f32)
                idx = 0
                for j in range(NB):
                    for c in range(N_CHUNK):
                        sl = prods[c][:, j * D:(j + 1) * D]
                        col = e_cols[:, idx:idx + 1]
                        if idx % 16 < RED_SPLIT:
                            nc.vector.reduce_sum(
                                out=col, in_=sl, axis=mybir.AxisListType.X)
                        else:
                            adum = scrp.tile([P, D], bf16, tag="adum")
                            nc.scalar.activation(
                                out=adum, in_=sl, func=Act.Copy,
                                accum_out=col)
                        idx += 1

                # phase 3: p = exp(tanh(e + hb[b])), p in bf16 for the PE
                p_ts = []
                prs_g = small.tile([P, NB], f32)
                for j in range(NB):
                    b = g * NB + j
                    t_t = small.tile([P, N_CHUNK], f32)
                    nc.scalar.activation(
                        out=t_t,
                        in_=e_cols[:, j * N_CHUNK:(j + 1) * N_CHUNK],
                        func=Act.Tanh,
                        bias=hb_t[:, b:b + 1])
                    p_t = small.tile([P, N_CHUNK], bf16)
                    nc.scalar.activation(out=p_t, in_=t_t, func=Act.Exp)
                    p_ts.append(p_t)
                    nc.vector.reduce_sum(out=prs_g[:, j:j + 1], in_=p_t,
                                         axis=mybir.AxisListType.X)

                # phase 4: denominators for the whole group in one matmul
                den_ps = pden.tile([1, NB], f32)
                nc.tensor.matmul(out=den_ps, lhsT=ones_t, rhs=prs_g,
                                 start=True, stop=True)
                rec_g = small.tile([1, NB], f32)
                nc.vector.reciprocal(out=rec_g, in_=den_ps)

                # phase 5: context matmuls, dense burst on the PE
                ctx_pss = []
                for j in range(NB):
                    ctx_ps = pctx.tile([1, D], f32)
                    for c in range(N_CHUNK):
                        nc.tensor.matmul(
                            out=ctx_ps,
                            lhsT=p_ts[j][:, c:c + 1],
                            rhs=enc_tiles[c][:, j * D:(j + 1) * D],
                            start=(c == 0),
                            stop=(c == N_CHUNK - 1),
                        )
                    ctx_pss.append(ctx_ps)
                for j in range(NB):
                    nc.scalar.activation(
                        out=out_t[:, j * D:(j + 1) * D],
                        in_=ctx_pss[j],
                        func=Act.Copy,
                        scale=rec_g[:, j:j + 1],
                    )

                nc.sync.dma_start(
                    out=out[g * NB:(g + 1) * NB, :],
                    in_=out_t,
                )

    nc.compile()
    return nc


def _get_nc(b_loc=B_LOC):
    if b_loc not in _NC_CACHE:
        _NC_CACHE[b_loc] = _build_nc(b_loc)
    return _NC_CACHE[b_loc]


def _host_prep(encoder_outputs, hidden_state, W, b):
    import ml_dtypes

    enc = np.asarray(encoder_outputs, dtype=np.float32)
    h = np.asarray(hidden_state, dtype=np.float32)
    W = np.asarray(W, dtype=np.float32)
    b = np.asarray(b, dtype=np.float32)
    # device-side compute streams enc in bf16 (round-to-nearest); the
    # softmax weights and normalization stay fp32 on device
    enc_bf = enc.astype(ml_dtypes.bfloat16)
    We = np.tile(np.ascontiguousarray(W[:D, 0]), NB).astype(ml_dtypes.bfloat16)
    Wh = W[D:, 0]
    hb = (h @ Wh + b[0]).astype(np.float32)  # (S,)
    return enc_bf, We, hb


def kernel(encoder_outputs, hidden_state, W, b):
    enc, We, hb = _host_prep(encoder_outputs, hidden_state, W, b)
    nc = _get_nc()

    from concourse import bass_utils

    in_maps = []
    for c in range(N_CORES):
        sl = slice(c * B_LOC, (c + 1) * B_LOC)
        in_maps.append({
            "enc": np.ascontiguousarray(enc[:, sl, :]),
            "web": We,
            "hbb": np.ascontiguousarray(hb[sl]),
        })

    res = bass_utils.run_bass_kernel_spmd(
        nc,
        in_maps,
        core_ids=list(range(N_CORES)),
        trace=bool(int(os.environ.get("KERNEL_TRACE", "0"))),
    )
    global LAST_RESULTS
    LAST_RESULTS = res
    ctx = np.concatenate([r["out"] for r in res.results], axis=0)  # (S, D)
    return ctx[None].astype(np.float32)


LAST_RESULTS = None
